# revision 6
# baseline (speedup 1.0000x reference)
"""BinaryMoSLinear Trainium2 kernel (8-core SPMD, data-parallel over tokens).

Math (per reference):
    xf      = x.reshape(N, H)
    routing = softmax(xf @ gate_w.T)            # [N, E], E = 8
    in_s    = routing @ in_channel_scale        # [N, H]
    out_s   = routing @ out_channel_scale       # [N, O]
    out     = (xf * in_s) @ sign(weight).T * out_s + bias

Device factorization (division-free, all matmuls contract on partitions):
    expT[e, t]   = exp(logitsT[e, t])          (raw, unstabilized; bf16)
    den[t]       = sum_e expT[e, t]            (PE mm with a ones column)
    is_raw[h, t] = sum_e ics[e, h] expT[e, t]  (PE mm, natural ics layout)
    aT[h, t]     = bf16(xT[h, t] * is_raw)     (softmax denom factored out)
    main[t, o]   = sum_h aT[h, t] sign(w)[o, h]
    os_raw[t, o] = sum_e expT[e, t] ocs[e, o]
    out[t, o]    = main * os_raw / den[t]^2 + bias[o]

Each core gets 1024 tokens and the full weight; no collectives.  x and the
binarized weight are transposed on-chip with PE transposes (bf16; sign(w) is
exact in bf16).  The 1/den^2 factor is applied in natural orientation where
t sits on partitions, so it is a per-partition tensor_scalar.
"""

import numpy as np

import concourse.bass as bass
import concourse.mybir as mybir
from concourse import tile
from concourse.bass_utils import run_bass_kernel_spmd
from concourse.masks import make_identity

F32 = mybir.dt.float32
BF16 = mybir.dt.bfloat16
AF = mybir.ActivationFunctionType
ALU = mybir.AluOpType

P = 128
E = 8
N_CORES = 8

# full problem: x [4, 2048, 4096], weight [4096, 4096]
FULL_B, FULL_S, FULL_H, FULL_O = 4, 2048, 4096, 4096
TOK = FULL_B * FULL_S // N_CORES  # 1024 tokens per core


# --------------------------------------------------------------------------
# This container's walrus build accepts at most ONE sync-wait command per
# instruction (DMA descriptors especially).  Tile's scheduler freely stacks
# several waits on one instruction, so rewrite the BIR JSON before compile:
# excess waits become single-wait NoOps immediately preceding the instruction
# on the same engine (program order => identical semantics).
_MAXW = 1


def _split_excess_waits(bir_json: bytes, maxw: int = _MAXW) -> bytes:
    import json as _json

    j = _json.loads(bir_json)
    ctr = 0
    for fn in j["functions"]:
        for blk in fn["blocks"]:
            new = []
            for inst in blk["instructions"]:
                si = inst.get("sync_info")
                if si:
                    waits = si.get("on_wait") or []
                    if len(waits) > maxw:
                        extra, keep = waits[:-maxw], waits[-maxw:]
                        for i in range(0, len(extra), maxw):
                            ctr += 1
                            nop = {
                                "name": f"I-wsplit-{ctr}",
                                "opcode": "NoOp",
                                "engine": inst["engine"],
                                "ins": [],
                                "outs": [],
                                "sync_info": {
                                    "on_wait": extra[i : i + maxw],
                                    "on_update": [],
                                },
                            }
                            if "debug" in inst:
                                nop["debug"] = inst["debug"]
                            new.append(nop)
                        si["on_wait"] = keep
                new.append(inst)
            blk["instructions"] = new
    return _json.dumps(j).encode()


def _install_wait_split():
    from concourse import bass2jax, bass_utils

    orig = bass_utils.compile_bir_kernel
    if getattr(orig, "_wait_split_wrapped", False):
        return

    def wrapped(bir_json, tmpdir, neff_name="file.neff"):
        return orig(_split_excess_waits(bir_json), tmpdir, neff_name)

    wrapped._wait_split_wrapped = True
    bass_utils.compile_bir_kernel = wrapped
    bass2jax.compile_bir_kernel = wrapped


_install_wait_split()
# --------------------------------------------------------------------------


def build_nc(tok=TOK, h=FULL_H, o=FULL_O):
    """Build the per-core Bass program.  tok/h/o shrinkable for debugging."""
    HC = h // P          # 128-wide h-chunks
    TB = tok // P        # 128-token blocks
    TH = tok // 512      # 512-token halves (in_scale granularity)
    OC = o // 512        # 512-wide output chunks
    JH = 16              # h-chunks per wbT pipeline stage
    HH = HC // JH        # wbT stages per output chunk
    assert tok % 512 == 0 and h % (JH * P) == 0 and o % 512 == 0

    nc = bass.Bass("TRN2", target_bir_lowering=False, debug=False,
                   num_devices=N_CORES)

    x_d = nc.declare_dram_parameter("x", [tok, h], F32, isOutput=False)
    w_d = nc.declare_dram_parameter("weight", [o, h], F32, isOutput=False)
    b_d = nc.declare_dram_parameter("bias", [o], F32, isOutput=False)
    gw_d = nc.declare_dram_parameter("gate_w", [E, h], F32, isOutput=False)
    ics_d = nc.declare_dram_parameter("ics", [E, h], F32, isOutput=False)
    ocs_d = nc.declare_dram_parameter("ocs", [E, o], F32, isOutput=False)
    out_d = nc.declare_dram_parameter("out", [tok, o], F32, isOutput=True)

    with tile.TileContext(nc) as tc:
        with (
            tc.tile_pool(name="const", bufs=1) as const,
            tc.tile_pool(name="sb", bufs=2) as sb,
            tc.tile_pool(name="wsgn", bufs=4) as wsgnp,
            tc.tile_pool(name="wbt", bufs=3) as wbtp,
            tc.tile_pool(name="pmm", bufs=4, space="PSUM") as pmm,
            tc.tile_pool(name="pos", bufs=1, space="PSUM") as posp,
            tc.tile_pool(name="pT", bufs=2, space="PSUM") as pT,
            tc.tile_pool(name="psmall", bufs=1, space="PSUM") as psmall,
        ):
            # ---- constants / persistent tiles ----
            id_bf = const.tile([P, P], BF16, name="id_bf")
            make_identity(nc, id_bf)
            ones_bf = const.tile([P, 1], BF16, name="ones_bf")
            nc.gpsimd.memset(ones_bf, 1.0)

            aT = const.tile([P, HC * tok], BF16, name="aT")
            aT3 = aT.rearrange("p (hc t) -> p hc t", t=tok)
            expT = const.tile([P, tok], BF16, name="expT")
            nc.gpsimd.memset(expT, 0.0)
            invden = const.tile([P, TB], F32, name="invden")
            invden2 = const.tile([P, TB], F32, name="invden2")

            gwT = const.tile([P, HC * E], BF16, name="gwT")
            ics_bf = const.tile([P, h], BF16, name="ics_bf")
            nc.gpsimd.memset(ics_bf, 0.0)
            nc.gpsimd.dma_start(out=ics_bf[0:E, :], in_=ics_d[:, :])
            ocs_bf = const.tile([P, o], BF16, name="ocs_bf")
            nc.gpsimd.memset(ocs_bf, 0.0)
            nc.gpsimd.dma_start(out=ocs_bf[0:E, :], in_=ocs_d[:, :])
            bias_bc = const.tile([P, o], BF16, name="bias_bc")
            nc.gpsimd.dma_start(
                out=bias_bc, in_=b_d[None, :].to_broadcast((P, o))
            )

            # gwT: transpose gate_w (zero-padded to 128 partitions)
            gw_bf = sb.tile([P, h], BF16, tag="xbf")
            nc.gpsimd.memset(gw_bf, 0.0)
            nc.gpsimd.dma_start(out=gw_bf[0:E, :], in_=gw_d[:, :])
            for hc in range(HC):
                pt = pT.tile([P, 512], BF16, tag="T4")
                nc.tensor.transpose(
                    pt[:, 0:P], gw_bf[:, hc * P : (hc + 1) * P], id_bf
                )
                nc.vector.tensor_copy(
                    out=gwT[:, hc * E : (hc + 1) * E], in_=pt[:, 0:E]
                )

            # ---- phase A: x -> xT -> gating -> aT (scaled, bf16) ----
            for tb in range(TB):
                t0 = tb * P
                x_bf = sb.tile([P, h], BF16, tag="xbf")
                nc.gpsimd.dma_start(out=x_bf, in_=x_d[t0 : t0 + P, :])
                for j0 in range(0, HC, 4):
                    pt = pT.tile([P, 512], BF16, tag="T4")
                    for k in range(4):
                        nc.tensor.transpose(
                            pt[:, k * P : (k + 1) * P],
                            x_bf[:, (j0 + k) * P : (j0 + k + 1) * P],
                            id_bf,
                        )
                    nc.vector.tensor_copy(
                        out=aT3[:, j0 : j0 + 4, t0 : t0 + P],
                        in_=pt.rearrange("p (b t) -> p b t", t=P),
                    )
                # logitsT [E, 128] = sum_hc gwT_chunk.T @ xT_chunk
                pl = psmall.tile([E, P], F32, tag="small")
                for hc in range(HC):
                    nc.tensor.matmul(
                        pl,
                        gwT[:, hc * E : (hc + 1) * E],
                        aT[:, hc * tok + t0 : hc * tok + t0 + P],
                        start=(hc == 0),
                        stop=(hc == HC - 1),
                    )
                nc.scalar.activation(expT[0:E, t0 : t0 + P], pl, AF.Exp)
                # den[t] (t on partitions): expT_chunk.T @ ones
                pd = psmall.tile([P, 1], F32, tag="small")
                nc.tensor.matmul(
                    pd, expT[:, t0 : t0 + P], ones_bf, start=True, stop=True
                )
                nc.vector.reciprocal(invden[:, tb : tb + 1], pd)
                nc.vector.tensor_tensor(
                    invden2[:, tb : tb + 1],
                    invden[:, tb : tb + 1],
                    invden[:, tb : tb + 1],
                    ALU.mult,
                )

                # after each 512-token half: fold is_raw into aT
                if (tb + 1) % 4 == 0:
                    th = tb // 4
                    s0 = th * 512
                    for hc in range(HC):
                        pis = pmm.tile([P, 512], F32, tag="mm")
                        nc.tensor.matmul(
                            pis,
                            ics_bf[:, hc * P : (hc + 1) * P],
                            expT[:, s0 : s0 + 512],
                            start=True,
                            stop=True,
                        )
                        sl = aT[:, hc * tok + s0 : hc * tok + s0 + 512]
                        nc.vector.tensor_tensor(sl, sl, pis, ALU.mult)

            # ---- phase C: weight sign+transpose + main matmul + epilogue ----
            for oc in range(OC):
                o0 = oc * 512
                wbt = []
                for hh in range(HH):
                    wt = wbtp.tile([P, JH * 512], BF16, tag="wbt")
                    wbt.append(wt)
                    wsgn = []
                    for s in range(4):
                        wf = sb.tile([P, JH * P], F32, tag="wf32")
                        nc.sync.dma_start(
                            out=wf,
                            in_=w_d[
                                o0 + s * P : o0 + (s + 1) * P,
                                hh * JH * P : (hh + 1) * JH * P,
                            ],
                        )
                        ws = wsgnp.tile([P, JH * P], BF16, tag="wsgn")
                        nc.scalar.activation(ws, wf, AF.Sign)
                        wsgn.append(ws)
                    for j in range(JH):
                        pt = pT.tile([P, 512], BF16, tag="T4")
                        for s in range(4):
                            nc.tensor.transpose(
                                pt[:, s * P : (s + 1) * P],
                                wsgn[s][:, j * P : (j + 1) * P],
                                id_bf,
                            )
                        nc.vector.tensor_copy(
                            out=wt[:, j * 512 : (j + 1) * 512], in_=pt
                        )

                for tg in range(TB // 4):
                    tbs = list(range(tg * 4, tg * 4 + 4))
                    pms = [pmm.tile([P, 512], F32, tag="mm", name=f"pm_{oc}_{tg}_{i}") for i in range(len(tbs))]
                    for hh in range(HH):
                        for i, tb in enumerate(tbs):
                            t0 = tb * P
                            for j in range(JH):
                                hc = hh * JH + j
                                nc.tensor.matmul(
                                    pms[i],
                                    aT[:, hc * tok + t0 : hc * tok + t0 + P],
                                    wbt[hh][:, j * 512 : (j + 1) * 512],
                                    start=(hh == 0 and j == 0),
                                    stop=(hh == HH - 1 and j == JH - 1),
                                )
                    for i, tb in enumerate(tbs):
                        t0 = tb * P
                        pos = posp.tile([P, 512], F32, tag="os")
                        nc.tensor.matmul(
                            pos,
                            expT[:, t0 : t0 + P],
                            ocs_bf[:, o0 : o0 + 512],
                            start=True,
                            stop=True,
                        )
                        tmp = sb.tile([P, 512], F32, tag="out", bufs=3)
                        # evacuate psum_main via ACT, folding in 1/den^2
                        # (per-partition scale); avoids a 2-PSUM-input DVE op
                        nc.scalar.activation(
                            tmp, pms[i], AF.Copy,
                            scale=invden2[:, tb : tb + 1],
                        )
                        nc.vector.tensor_tensor(tmp, tmp, pos, ALU.mult)
                        nc.vector.tensor_tensor(
                            tmp, tmp, bias_bc[:, o0 : o0 + 512], ALU.add
                        )
                        nc.sync.dma_start(
                            out=out_d[t0 : t0 + P, o0 : o0 + 512], in_=tmp
                        )
    return nc


_NC_CACHE = {}


def _get_nc(key=(TOK, FULL_H, FULL_O)):
    if key not in _NC_CACHE:
        _NC_CACHE[key] = build_nc(*key)
    return _NC_CACHE[key]


def kernel(x, weight, bias, gate_w, in_channel_scale, out_channel_scale):
    B, S, H = x.shape
    xf = np.ascontiguousarray(x.reshape(-1, H).astype(np.float32, copy=False))
    weight = np.ascontiguousarray(weight.astype(np.float32, copy=False))
    bias = np.ascontiguousarray(bias.astype(np.float32, copy=False))
    gate_w = np.ascontiguousarray(gate_w.astype(np.float32, copy=False))
    ics = np.ascontiguousarray(in_channel_scale.astype(np.float32, copy=False))
    ocs = np.ascontiguousarray(out_channel_scale.astype(np.float32, copy=False))

    nc = _get_nc()
    in_maps = [
        {
            "x": xf[c * TOK : (c + 1) * TOK],
            "weight": weight,
            "bias": bias,
            "gate_w": gate_w,
            "ics": ics,
            "ocs": ocs,
        }
        for c in range(N_CORES)
    ]
    res = run_bass_kernel_spmd(nc, in_maps, list(range(N_CORES)))
    out = np.concatenate(
        [res.results[c]["out"] for c in range(N_CORES)], axis=0
    )
    return out.reshape(B, S, -1)
